# revision 1
# baseline (speedup 1.0000x reference)
"""Multi-head attention (dense transformer block) for 8 Trainium2 NeuronCores.

Problem: x [4, 2048, 1024] f32, w_qkv [3072, 1024], w_out [1024, 1024]
  qkv = x @ w_qkv.T ; split q,k,v ; 16 heads x 64 dims
  out = softmax(q k^T / 8) v ; y = out @ w_out.T

Sharding: 8 shards = (batch b in 0..3) x (head-half hh in 0..1).
Each core handles one batch and 8 heads end-to-end: QKV projection
column-split, attention for its 8 heads, out-projection row-split ->
partial y. Host sums the two partial y's per batch. No collectives.

Kernel structure (engines run their instruction streams in order, so the
phases are emitted as a software pipeline over head pairs):

    qk(0) | v | B(0) qk(1) C(0) | B(1) qk(2) C(1) | ... | B(3) C(3)

  - qk(p): q^T,k^T [128, tok] for pair p (fp32r matmuls, rotating bufs)
  - v: value projection -> vaug bf16 [ktok, head, 65] with a ones column
  - B(p): attention. Scores computed transposed per head S^T[ktok, qtok]
    with the two heads PAIRED via PE row-tiling (K=64 at partitions
    0/64) into adjacent PSUM banks; one ScalarE exp ACTIVATE [128, 1024]
    per k-tile covers both heads with the 1/8 scale folded in (softmax
    max-subtraction skipped; scores are O(+-6)). AV matmuls in bf16 with
    M=65: the ones column makes PSUM row 64 the softmax denominators.
    Normalization: DVE reciprocal -> GpSimd partition-broadcast -> DVE
    multiply (PE stays out of the softmax epilogue).
  - C(p): per-pair out-projection (K=128), accumulated into y in DRAM
    (first pair writes, later pairs DMA-accumulate).
"""

import numpy as np

B = 4
NT = 2048          # tokens per batch
E = 1024           # embed dim
H = 16             # heads
DH = 64            # head dim
HD = 512           # head dims per core (8 heads)
N_CORES = 8
SCALE = DH ** -0.5
P = 128

_cache = {}


def _build(rep=1, ablate=(), mmdt="f32r", loop=False):
    import concourse.mybir as mybir
    import concourse.tile as tile
    from concourse import bacc
    from contextlib import ExitStack

    # dtype scheme: f32r/bf16/fp16 uniform; "mix" = fp16 q/k path + bf16 soft path
    f32 = mybir.dt.float32
    _qk = {"f32r": mybir.dt.float32r, "bf16": mybir.dt.bfloat16,
           "fp16": mybir.dt.float16, "mix": mybir.dt.float16}
    _soft = {"f32r": mybir.dt.bfloat16, "bf16": mybir.dt.bfloat16,
             "fp16": mybir.dt.float16, "mix": mybir.dt.bfloat16}
    f32r = _qk[mmdt]          # q/k-side matmul dtype (x, wq, wk, wv, qT, kT)
    bf16 = _soft[mmdt]        # softmax/out-side dtype (es, vaug, outT, woT)
    in_dt = {"f32r": f32, "bf16": mybir.dt.bfloat16,
             "fp16": mybir.dt.float16, "mix": mybir.dt.float16}[mmdt]
    wo_dt = {"f32r": f32, "bf16": mybir.dt.bfloat16,
             "fp16": mybir.dt.float16, "mix": mybir.dt.bfloat16}[mmdt]
    Exp = mybir.ActivationFunctionType.Exp
    Add = mybir.AluOpType.add

    nc = bacc.Bacc("TRN2", target_bir_lowering=False, debug=False,
                   enable_asserts=False, num_devices=N_CORES)

    xT_ap = nc.dram_tensor("xT", [E, NT], in_dt, kind="ExternalInput").ap()
    wqT_ap = nc.dram_tensor("wqT", [E, HD], in_dt, kind="ExternalInput").ap()
    wkT_ap = nc.dram_tensor("wkT", [E, HD], in_dt, kind="ExternalInput").ap()
    wvT_ap = nc.dram_tensor("wvT", [E, HD], in_dt, kind="ExternalInput").ap()
    woT_ap = nc.dram_tensor("woT", [HD, E], wo_dt, kind="ExternalInput").ap()
    y_ap = nc.dram_tensor("y", [NT, E], f32, kind="ExternalOutput").ap()

    KE = E // P        # 8 contraction tiles over embed
    MQ = HD // P       # 4 partition tiles over head dims = head pairs
    TQ = NT // 512     # 4 query chunks of 512
    TT = NT // P       # 16 token tiles of 128

    from concourse.tile_rust import add_dep_helper

    with tile.TileContext(nc) as tc, ExitStack() as ctx:
        per = ctx.enter_context(tc.tile_pool(name="per", bufs=1))
        qk_pool = ctx.enter_context(tc.tile_pool(name="qk", bufs=2))
        outT_pool = ctx.enter_context(tc.tile_pool(name="ot", bufs=2))
        es_pool = ctx.enter_context(tc.tile_pool(name="es", bufs=3))
        y_pool = ctx.enter_context(tc.tile_pool(name="ysb", bufs=2))
        nrm_pool = ctx.enter_context(tc.tile_pool(name="nrm", bufs=1))
        bcs_pool = ctx.enter_context(tc.tile_pool(name="bcs", bufs=1))
        xT_pool = ctx.enter_context(tc.tile_pool(name="xTp", bufs=1))
        psS = ctx.enter_context(tc.tile_pool(name="psS", bufs=2, space="PSUM"))
        psAV = ctx.enter_context(tc.tile_pool(name="psAV", bufs=2, space="PSUM"))
        psM = ctx.enter_context(tc.tile_pool(name="psM", bufs=2, space="PSUM"))

        # rep-invariant weights (wv first: the value projection runs first)
        wv = per.tile([P, KE, HD], f32r, tag="wv")
        nc.scalar.dma_start(wv[:], wvT_ap.rearrange("(o p) m -> p o m", p=P).bitcast(f32r))
        wq = per.tile([P, KE, HD], f32r, tag="wq")
        nc.scalar.dma_start(wq[:], wqT_ap.rearrange("(o p) m -> p o m", p=P).bitcast(f32r))
        wk = per.tile([P, KE, HD], f32r, tag="wk")
        nc.scalar.dma_start(wk[:], wkT_ap.rearrange("(o p) m -> p o m", p=P).bitcast(f32r))
        woT = per.tile([P, MQ, E], bf16, tag="woT")
        nc.scalar.dma_start(woT[:], woT_ap.rearrange("(o p) e -> p o e", p=P).bitcast(bf16))
        vaug_g = [per.tile([P, 4, 8, DH + 1], bf16, tag=f"vaug{g}", name=f"vaug{g}")
                  for g in range(TT // 4)]
        vaugs = [vaug_g[tt // 4][:, tt % 4] for tt in range(TT)]
        for g in range(TT // 4):
            nc.vector.memset(vaug_g[g][:, :, :, DH:DH + 1], 1.0)

        # Tile does not order DMAs by DRAM range: chain each y region's
        # write/accumulate DMAs explicitly (across pairs and reps).
        y_prev_dma = {}

        def emit_body():
            xTs = []
            xT_src = xT_ap.rearrange("(o p) t -> p o t", p=P).bitcast(f32r)
            for ke in range(KE):
                xk = xT_pool.tile([P, NT], f32r, tag=f"xT{ke}", name=f"xT{ke}")
                nc.sync.dma_start(xk[:], xT_src[:, ke, :])
                xTs.append(xk)

            def emit_qk_group(mq, dst, w, tq, rot=0):
                ps = psM.tile([P, 512], f32, tag="m")
                for i in range(KE):
                    ke = (i + rot) % KE
                    nc.tensor.matmul(ps[:], w[:, ke, mq * P:(mq + 1) * P],
                                     xTs[ke][:, tq * 512:(tq + 1) * 512],
                                     start=(i == 0), stop=(i == KE - 1))
                nc.vector.tensor_copy(dst[:, tq * 512:(tq + 1) * 512], ps[:])

            def alloc_qk(mq):
                qT = qk_pool.tile([P, NT], f32r, tag="qTp", name=f"qT{mq}")
                kT = qk_pool.tile([P, NT], f32r, tag="kTp", name=f"kT{mq}")
                return qT, kT

            def qk_groups(mq, qT, kT):
                for dst, w in ((kT, wk), (qT, wq)):
                    for tq in range(TQ):
                        yield (mq, dst, w, tq)

            def emit_v():
                for tt in range(TT):
                    ps = psM.tile([P, HD], f32, tag="m")
                    for i in range(KE):
                        ke = (i + tt) % KE
                        nc.tensor.matmul(ps[:], xTs[ke][:, tt * P:(tt + 1) * P],
                                         wv[:, ke, :], start=(i == 0), stop=(i == KE - 1))
                    nc.scalar.copy(vaugs[tt][:, :, 0:DH],
                                   ps[:].rearrange("p (h d) -> p h d", h=8))

            def emit_attn_tq(pair, qT, kT, outT, tq):
                qsl = slice(tq * 512, (tq + 1) * 512)
                av0 = psAV.tile([DH + 1, 512], f32, tag="av")
                av1 = psAV.tile([DH + 1, 512], f32, tag="av")

                def emit_av(kt, es):
                    nc.tensor.matmul(av0[:], vaugs[kt][:, 2 * pair, :], es[:, 0, :],
                                     start=(kt == 0), stop=(kt == TT - 1))
                    nc.tensor.matmul(av1[:], vaugs[kt][:, 2 * pair + 1, :], es[:, 1, :],
                                     start=(kt == 0), stop=(kt == TT - 1))

                # AV lags scores/exp by one k-tile so the PE never sits in
                # the scores -> exp -> AV semaphore chain: while ScalarE
                # exps tile kt, the PE already runs scores of kt+1.
                pending = None
                for kt in range(TT):
                    ksl = slice(kt * P, (kt + 1) * P)
                    sps = psS.tile([P, 2, 512], f32, tag="s")
                    nc.tensor.matmul(sps[:, 0, :], kT[0:DH, ksl],
                                     qT[0:DH, qsl], start=True, stop=True)
                    nc.tensor.matmul(sps[:, 1, :], kT[DH:P, ksl],
                                     qT[DH:P, qsl], start=True, stop=True)
                    if "exp" in ablate:
                        continue
                    es = es_pool.tile([P, 2, 512], bf16, tag="es")
                    nc.scalar.activation(es[:], sps[:], Exp, scale=SCALE)
                    if "av" in ablate:
                        continue
                    if pending is not None:
                        emit_av(*pending)
                    pending = (kt, es)
                if "av" not in ablate and "exp" not in ablate:
                    emit_av(*pending)
                if "av" in ablate or "exp" in ablate:
                    return
                for j, av in ((0, av0), (1, av1)):
                    recip = nrm_pool.tile([1, 512], f32, tag="recip")
                    nc.vector.reciprocal(recip[:], av[DH:DH + 1, :])
                    bcs = bcs_pool.tile([DH, 512], f32, tag="bcs")
                    nc.gpsimd.partition_broadcast(bcs[:], recip[:])
                    nc.vector.tensor_tensor(
                        outT[j * DH:(j + 1) * DH, qsl],
                        av[0:DH, :], bcs[:], mybir.AluOpType.mult)

            def outproj_chunks(pair, outT):
                # y (+)= outT(pair).T @ woT[pair]; DRAM-side accumulation
                for tt in range(TT):
                    for ec in range(E // 512):
                        yield (pair, outT, tt, ec)

            def emit_outproj_chunk(pair, outT_a, outT_b, tt, ec):
                # two pairs' contributions accumulated in PSUM, then one
                # write (first half) or DMA-accumulate (second half)
                esl = slice(ec * 512, (ec + 1) * 512)
                ps = psAV.tile([P, 512], f32, tag="av")
                nc.tensor.matmul(ps[:], outT_a[:, tt * P:(tt + 1) * P],
                                 woT[:, pair - 1, esl], start=True, stop=False)
                nc.tensor.matmul(ps[:], outT_b[:, tt * P:(tt + 1) * P],
                                 woT[:, pair, esl], start=False, stop=True)
                ysb = y_pool.tile([P, 512], f32, tag="ysb")
                nc.vector.tensor_copy(ysb[:], ps[:])
                if pair == 1:
                    dma = nc.sync.dma_start(y_ap[tt * P:(tt + 1) * P, esl], ysb[:])
                else:
                    dma = nc.gpsimd.dma_start(y_ap[tt * P:(tt + 1) * P, esl],
                                              ysb[:], accum_op=Add)
                if (tt, ec) in y_prev_dma:
                    add_dep_helper(dma.ins, y_prev_dma[(tt, ec)].ins,
                                   reason="y accumulation order")
                y_prev_dma[(tt, ec)] = dma

            def drain(it, n):
                for _ in range(n):
                    args = next(it, None)
                    if args is None:
                        return
                    if len(args) == 4 and isinstance(args[0], int) and args[0] < MQ and not hasattr(args[1], "shape"):
                        emit_qk_group(*args)
                    else:
                        emit_outproj_chunk(*args)

            # software pipeline over head pairs:
            #   v | qk(0) | B(0)+qk(1)+C(0) | B(1)+qk(2)+C(1) | ... | B(3)+C(3)
            emit_v()
            qT, kT = alloc_qk(0)
            for gi, g in enumerate(qk_groups(0, qT, kT)):
                emit_qk_group(*g, rot=gi)
            prev_outT = None
            for pair in range(MQ):
                outT = outT_pool.tile([P, NT], bf16, tag="outT", name=f"outT{pair}")
                if pair + 1 < MQ:
                    nqT, nkT = alloc_qk(pair + 1)
                    qk_iter = qk_groups(pair + 1, nqT, nkT)
                else:
                    nqT = nkT = None
                    qk_iter = iter(())
                for tq in range(TQ):
                    if "scores" not in ablate:
                        emit_attn_tq(pair, qT, kT, outT, tq)
                    for _ in range(2):
                        g = next(qk_iter, None)
                        if g is not None:
                            emit_qk_group(*g)
                    if "outproj" in ablate or pair % 2 == 0:
                        continue
                    for tt in range(tq * 4, tq * 4 + 4):
                        for ec in range(E // 512):
                            emit_outproj_chunk(pair, prev_outT, outT, tt, ec)
                prev_outT = outT
                qT, kT = nqT, nkT

        if loop:
            with tc.For_i(0, rep, 1):
                emit_body()
        else:
            for _ in range(rep):
                emit_body()

    nc.compile()
    return nc


MMDT = "bf16"


def _get_nc(rep=1, ablate=(), mmdt=None):
    mmdt = mmdt or MMDT
    key = ("nc", rep, tuple(sorted(ablate)), mmdt)
    if key not in _cache:
        _cache[key] = _build(rep, ablate, mmdt)
    return _cache[key]


def make_in_maps(x, w_qkv, w_out, mmdt=None):
    import ml_dtypes
    mmdt = mmdt or MMDT
    dt = {"f32r": np.float32, "bf16": ml_dtypes.bfloat16,
          "fp16": np.float16, "mix": np.float16}[mmdt]
    wo_np = {"f32r": np.float32, "bf16": ml_dtypes.bfloat16,
             "fp16": np.float16, "mix": ml_dtypes.bfloat16}[mmdt]
    x = np.asarray(x, dtype=np.float32).astype(dt)
    w_qkv = np.asarray(w_qkv, dtype=np.float32).astype(dt)
    w_out = np.asarray(w_out, dtype=np.float32).astype(wo_np)
    in_maps = []
    for c in range(N_CORES):
        b, hh = divmod(c, 2)
        hsl = slice(hh * HD, (hh + 1) * HD)
        in_maps.append({
            "xT": np.ascontiguousarray(x[b].T),
            "wqT": np.ascontiguousarray(w_qkv[0 * E:1 * E][hsl].T),
            "wkT": np.ascontiguousarray(w_qkv[1 * E:2 * E][hsl].T),
            "wvT": np.ascontiguousarray(w_qkv[2 * E:3 * E][hsl].T),
            "woT": np.ascontiguousarray(w_out[:, hsl].T),
        })
    return in_maps


def combine_outputs(results):
    y = np.empty((B, NT, E), dtype=np.float32)
    for b in range(B):
        y[b] = results[2 * b]["y"] + results[2 * b + 1]["y"]
    return y


def kernel(x, w_qkv, w_out):
    from concourse.bass_utils import run_bass_kernel_spmd
    nc = _get_nc()
    in_maps = make_in_maps(x, w_qkv, w_out)
    res = run_bass_kernel_spmd(nc, in_maps, core_ids=list(range(N_CORES)))
    return combine_outputs(res.results)



# revision 42
# speedup vs baseline: 3.8941x; 3.8941x over previous
"""Multi-head attention (dense transformer block) for 8 Trainium2 NeuronCores.

Problem: x [4, 2048, 1024] f32, w_qkv [3072, 1024], w_out [1024, 1024]
  qkv = x @ w_qkv.T ; split q,k,v ; 16 heads x 64 dims
  out = softmax(q k^T / 8) v ; y = out @ w_out.T

Sharding: 8 shards = (batch b in 0..3) x (head-half hh in 0..1).
Each core handles one batch and 8 heads end-to-end: QKV projection
column-split, attention for its 8 heads, out-projection row-split ->
partial y. Host sums the two partial y's per batch. No collectives.

Design (v3). Engine budget per rep/core at full clocks: PE 657k cols
(~274us @2.4GHz), ScalarE exp ~266us, DVE ~45us, Pool ~110us. The
kernel is a software pipeline that keeps PE and ScalarE concurrently
saturated, including ACROSS reps (rep r+1's prologue drains inside
rep r's last attention pair).

  - qk(p): q^T,k^T [128, tok] per head pair (bf16 matmuls; PSUM->SBUF
    copies on DVE — GPSIMD cannot access PSUM on TRN2).
  - v: value projection -> vaug bf16 [ktok, head, 65] with a ones
    column (double-buffered across reps); copies on DVE.
  - attention per (pair, tq=512 queries): per k-tile scores S^T
    [ktok, q] for both heads (two K=64 matmuls into one PSUM tile),
    one ScalarE exp ACTIVATE [128, 2x512] -> es bf16 (softmax
    max-subtraction skipped; scores are O(+-6); 1/8 scale folded in).
    AV is FLIPPED: es chunks [128 ktok, 128 q] are the stationary
    operand, vaug [128 ktok, 65] streams -> psum [128 q, 65] per
    (head, q-chunk); the ones column makes psum col 64 the softmax
    denominator. 520 PE cols/ktile instead of 1024. AV lags exp by 2
    k-tiles so the PE never waits on a recent exp.
  - epilogue per tq: DVE copies the 8 AV accumulators out of PSUM,
    reciprocal on col 64, per-partition tensor_scalar multiply ->
    o_sb [q, (head,d)] bf16; PE-transposes (into bf16 views of psM-ring
    tiles) + DVE copies assemble outT [hd, tok]. PSUM accumulate
    start/stop is per BANK: one start per bank, later chunks accumulate.
  - outproj: per 2 pairs (K=128 each), chunks deferred TWO tqs and
    spread inside later kt loops; pairs 0+1 write y, pairs 2+3 write
    y2 (separate tensors; host sums - DRAM read-modify-write
    accumulation proved racy).
"""

import numpy as np

B = 4
NT = 2048          # tokens per batch
E = 1024           # embed dim
H = 16             # heads
DH = 64            # head dim
HD = 512           # head dims per core (8 heads)
N_CORES = 8
SCALE = DH ** -0.5
P = 128

_cache = {}


def _build(rep=1, ablate=(), mmdt="bf16", loop=False):
    import concourse.mybir as mybir
    import concourse.tile as tile
    from concourse import bacc
    from contextlib import ExitStack
    from itertools import chain

    f32 = mybir.dt.float32
    bf16 = {"bf16": mybir.dt.bfloat16, "fp16": mybir.dt.float16}[mmdt]
    in_dt = bf16
    Exp = mybir.ActivationFunctionType.Exp
    Add = mybir.AluOpType.add
    Mult = mybir.AluOpType.mult

    nc = bacc.Bacc("TRN2", target_bir_lowering=False, debug=False,
                   enable_asserts=False, num_devices=N_CORES)

    xT_ap = nc.dram_tensor("xT", [E, NT], in_dt, kind="ExternalInput").ap()
    wqT_ap = nc.dram_tensor("wqT", [E, HD], in_dt, kind="ExternalInput").ap()
    wkT_ap = nc.dram_tensor("wkT", [E, HD], in_dt, kind="ExternalInput").ap()
    wvT_ap = nc.dram_tensor("wvT", [E, HD], in_dt, kind="ExternalInput").ap()
    woT_ap = nc.dram_tensor("woT", [HD, E], in_dt, kind="ExternalInput").ap()
    id_ap = nc.dram_tensor("ident", [P, P], in_dt, kind="ExternalInput").ap()
    y_ap = nc.dram_tensor("y", [NT, E], f32, kind="ExternalOutput").ap()
    # pairs 0+1 write y, pairs 2+3 write y2; host sums them. A DRAM
    # read-modify-write accumulate chained after the first write proved
    # racy (NaN from uninitialized DRAM) - two plain writes are ordering-free.
    y2_ap = nc.dram_tensor("y2", [NT, E], f32, kind="ExternalOutput").ap()

    KE = E // P        # 8 contraction tiles over embed
    MQ = HD // P       # 4 partition tiles over head dims = head pairs
    TQ = NT // 512     # 4 query chunks of 512
    TT = NT // P       # 16 token tiles of 128

    from concourse.tile_rust import add_dep_helper

    with tile.TileContext(nc) as tc, ExitStack() as ctx:
        per = ctx.enter_context(tc.tile_pool(name="per", bufs=1))
        qk_pool = ctx.enter_context(tc.tile_pool(name="qk", bufs=2))
        outT_pool = ctx.enter_context(tc.tile_pool(name="ot", bufs=3))
        es_pool = ctx.enter_context(tc.tile_pool(name="es", bufs=5))
        y_pool = ctx.enter_context(tc.tile_pool(name="ysb", bufs=4))
        avsb_pool = ctx.enter_context(tc.tile_pool(name="avsb", bufs=2))
        osb_pool = ctx.enter_context(tc.tile_pool(name="osb", bufs=2))
        nrm_pool = ctx.enter_context(tc.tile_pool(name="nrm", bufs=2))
        xT_pool = ctx.enter_context(tc.tile_pool(name="xTp", bufs=1))
        vaug_pool = ctx.enter_context(tc.tile_pool(name="vau", bufs=2))
        psS = ctx.enter_context(tc.tile_pool(name="psS", bufs=2, space="PSUM"))
        psA = ctx.enter_context(tc.tile_pool(name="psA", bufs=1, space="PSUM"))
        psM = ctx.enter_context(tc.tile_pool(name="psM", bufs=2, space="PSUM"))

        # rep-invariant weights
        wv = per.tile([P, KE, HD], in_dt, tag="wv")
        nc.scalar.dma_start(wv[:], wvT_ap.rearrange("(o p) m -> p o m", p=P))
        wq = per.tile([P, KE, HD], in_dt, tag="wq")
        nc.scalar.dma_start(wq[:], wqT_ap.rearrange("(o p) m -> p o m", p=P))
        wk = per.tile([P, KE, HD], in_dt, tag="wk")
        nc.scalar.dma_start(wk[:], wkT_ap.rearrange("(o p) m -> p o m", p=P))
        woT = per.tile([P, MQ, E], bf16, tag="woT")
        nc.scalar.dma_start(woT[:], woT_ap.rearrange("(o p) e -> p o e", p=P))
        ident = per.tile([P, P], bf16, tag="ident")
        nc.scalar.dma_start(ident[:], id_ap)

        # Tile does not order DMAs by DRAM range: chain each y region's
        # write/accumulate DMAs explicitly (across pairs and reps).
        y_prev_dma = {}
        # XBAR-transpose writes into outT are not reliably ordered against
        # readers by Tile: track the transpose instruction per (outT, tt)
        # and add explicit deps from the outproj matmuls that read it.
        tp_map = {}
        xT_src = xT_ap.rearrange("(o p) t -> p o t", p=P)

        def load_xT():
            xTs = []
            for ke in range(KE):
                xk = xT_pool.tile([P, NT], in_dt, tag=f"xT{ke}", name=f"xT{ke}")
                nc.sync.dma_start(xk[:], xT_src[:, ke, :])
                xTs.append(xk)
            return xTs

        def alloc_vaug():
            g = [vaug_pool.tile([P, 4, 8, DH + 1], bf16, tag=f"vaug{i}",
                                name=f"vaug{i}") for i in range(TT // 4)]
            for t in g:
                nc.vector.memset(t[:, :, :, DH:DH + 1], 1.0)
            return [g[tt // 4][:, tt % 4] for tt in range(TT)]

        def alloc_qk(mq):
            qT = qk_pool.tile([P, NT], in_dt, tag="qTp", name=f"qT{mq}")
            kT = qk_pool.tile([P, NT], in_dt, tag="kTp", name=f"kT{mq}")
            return qT, kT

        def emit_qk_group(xTs, mq, dst, w, tq, rot=0):
            ps = psM.tile([P, 512], f32, tag="m")
            for i in range(KE):
                ke = (i + rot) % KE
                nc.tensor.matmul(ps[:], w[:, ke, mq * P:(mq + 1) * P],
                                 xTs[ke][:, tq * 512:(tq + 1) * 512],
                                 start=(i == 0), stop=(i == KE - 1))
            nc.vector.tensor_copy(dst[:, tq * 512:(tq + 1) * 512], ps[:])

        def qk_groups(xTs, mq, qT, kT):
            # yielded as half-group closures (4 matmuls each) to cap the
            # PE-stream jitter a single drain slot can inject
            for dst, w in ((kT, wk), (qT, wq)):
                for tq in range(TQ):
                    ps_box = []

                    def mk(half, dst=dst, w=w, tq=tq, ps_box=ps_box):
                        def emit():
                            if half == 0:
                                ps_box.append(psM.tile([P, 512], f32, tag="m",
                                                       name="qkps"))
                            ps = ps_box[0]
                            for i in range(4 * half, 4 * half + 4):
                                nc.tensor.matmul(
                                    ps[:], w[:, i, mq * P:(mq + 1) * P],
                                    xTs[i][:, tq * 512:(tq + 1) * 512],
                                    start=(i == 0), stop=(i == KE - 1))
                            if half == 1:
                                nc.vector.tensor_copy(
                                    dst[:, tq * 512:(tq + 1) * 512], ps[:])
                        return emit

                    yield ("call", mk(0))
                    yield ("call", mk(1))

        def emit_v_tile(xTs, vaugs, tt):
            ps = psM.tile([P, HD], f32, tag="m")
            for i in range(KE):
                ke = (i + tt) % KE
                nc.tensor.matmul(ps[:], xTs[ke][:, tt * P:(tt + 1) * P],
                                 wv[:, ke, :], start=(i == 0), stop=(i == KE - 1))
            nc.vector.tensor_copy(vaugs[tt][:, :, 0:DH],
                                  ps[:].rearrange("p (h d) -> p h d", h=8))

        def v_items(xTs, vaugs):
            return [("v", xTs, vaugs, tt) for tt in range(TT)]

        def emit_outproj_chunk(pair, outT_a, outT_b, tt, ec):
            # two pairs' contributions accumulated in PSUM, then one
            # write (first half) or DMA-accumulate (second half)
            esl = slice(ec * 512, (ec + 1) * 512)
            ps = psM.tile([P, 512], f32, tag="m")
            nc.tensor.matmul(ps[:], outT_a[:, tt * P:(tt + 1) * P],
                             woT[:, pair - 1, esl], start=True, stop=False)
            nc.tensor.matmul(ps[:], outT_b[:, tt * P:(tt + 1) * P],
                             woT[:, pair, esl], start=False, stop=True)
            ysb = y_pool.tile([P, 512], f32, tag="ysb")
            nc.vector.tensor_copy(ysb[:], ps[:])
            if pair == 1:
                nc.sync.dma_start(y_ap[tt * P:(tt + 1) * P, esl], ysb[:])
            else:
                nc.gpsimd.dma_start(y2_ap[tt * P:(tt + 1) * P, esl], ysb[:])

        def drain(it, n):
            for _ in range(n):
                args = next(it, None)
                if args is None:
                    return
                if args[0] == "call":
                    args[1]()
                elif args[0] == "v":
                    emit_v_tile(*args[1:])
                else:
                    emit_outproj_chunk(*args[1:])

        def make_drains(slots):
            # One slot attempted per drain() call; empty slots fall
            # through so the quota shifts to later iterators.
            def gen():
                for it in slots:
                    item = next(it, None)
                    if item is not None:
                        yield item
            return gen()

        # The attention kt-pipeline is CONTINUOUS across tq/pair/rep
        # boundaries: pending AV closures carry over, so ScalarE's exp
        # stream never drains while the PE flushes/restarts a tile.
        # AV lags scores/exp by LAG k-tiles so the PE only consumes exps
        # that finished long ago (no PE<->ACT chain serialization).
        LAG = 2
        av_pend = []  # (emit_av_closure, epilogue_closure_or_None)
        last_avsb_copy = [None]  # previous epilogue's avs->av_sb DVE copy

        def pump_av(n):
            while len(av_pend) > n:
                fn, epi = av_pend.pop(0)
                fn()
                if epi is not None:
                    epi()

        def emit_attn_tq(pair, qT, kT, vaugs, outT, tq, drains):
            qsl = slice(tq * 512, (tq + 1) * 512)
            avs = psA.tile([P, 8, P], f32, tag="avs")

            def mk_av(kt, es):
                def emit():
                    if "av" in ablate:
                        return
                    first = None
                    for j in range(2):
                        for qc in range(4):
                            i = 4 * j + qc
                            # PSUM accumulation start/stop is per BANK
                            # (2KB = 4 chunks): one start zeroes the whole
                            # bank, later chunks must accumulate onto it
                            m = nc.tensor.matmul(
                                avs[:, i, 0:DH + 1],
                                es[:, j, qc * P:(qc + 1) * P],
                                vaugs[kt][:, 2 * pair + j, :],
                                start=(kt == 0 and i % 4 == 0),
                                stop=(kt == TT - 1 and i % 4 == 3),
                                skip_group_check=True)
                            first = first or m
                    if kt == 0 and last_avsb_copy[0] is not None:
                        # the single-buffered avs PSUM is recycled across
                        # tqs: the zeroing first write must wait for the
                        # previous epilogue's read (emitted after this
                        # tile was allocated, so Tile cannot see the WAR)
                        add_dep_helper(first.ins, last_avsb_copy[0].ins,
                                       reason="avs WAR vs prev epilogue")
                return emit

            def epilogue():
                # pull AV accumulators out of PSUM, normalize,
                # DMA-transpose into outT [hd, tok]
                if "av" in ablate or "exp" in ablate:
                    return
                av_sb = avsb_pool.tile([P, 8, DH + 1], f32, tag="avsb",
                                       name="av_sb")
                last_avsb_copy[0] = nc.vector.tensor_copy(
                    av_sb[:], avs[:, :, 0:DH + 1])
                r8 = nrm_pool.tile([P, 8], f32, tag="r8", name="r8")
                nc.vector.reciprocal(r8[:], av_sb[:, :, DH:DH + 1])
                o_sb = osb_pool.tile([P, 4, 2, DH], bf16, tag="osb",
                                     name="o_sb")
                for j in range(2):
                    for qc in range(4):
                        i = 4 * j + qc
                        nc.vector.tensor_scalar(
                            o_sb[:, qc, j, :], av_sb[:, i, 0:DH],
                            r8[:, i:i + 1], None, Mult)
                for qc in range(4):
                    # PE transpose into a bf16 view of a psM-ring tile, then
                    # DVE copy into outT. (An XBAR dma_start_transpose here
                    # proved racy: PE consumers ran before the DMA landed
                    # despite the semaphore dep.)
                    mt = psM.tile([P, 512], f32, tag="m", name="tpp")
                    tp = mt[:, 0:P // 2].bitcast(bf16)
                    nc.tensor.transpose(tp, o_sb[:, qc].rearrange(
                        "p a b -> p (a b)"), ident[:])
                    nc.vector.tensor_copy(
                        outT[:, tq * 512 + qc * P: tq * 512 + (qc + 1) * P],
                        tp)

            for kt in range(TT):
                # drains FIRST: if scores(kt) must stall on the psS ring,
                # the independent drain work has already been dispatched
                # (in-order PE stream would otherwise jam it behind)
                drain(drains, 1)
                ksl = slice(kt * P, (kt + 1) * P)
                sps = psS.tile([P, 2, 512], f32, tag="s")
                nc.tensor.matmul(sps[:, 0, :], kT[0:DH, ksl],
                                 qT[0:DH, qsl], start=True, stop=True)
                nc.tensor.matmul(sps[:, 1, :], kT[DH:P, ksl],
                                 qT[DH:P, qsl], start=True, stop=True)
                if "exp" not in ablate:
                    es = es_pool.tile([P, 2, 512], bf16, tag="es")
                    nc.scalar.activation(es[:], sps[:], Exp, scale=SCALE)
                    av_pend.append(
                        (mk_av(kt, es), epilogue if kt == TT - 1 else None))
                    pump_av(LAG)

        # Outproj chunks are deferred TWO tqs so the outT DMA-transposes
        # they read have long completed (a 1-tq lag stalls the PE waiting
        # on the previous epilogue's XBAR transposes). FIFO keyed by the
        # global tq index at which each batch of chunks was produced.
        import collections as _c
        op_new = _c.deque()     # (gtq_produced, [items...])
        op_avail = _c.deque()   # flat, ready items
        gtq_box = [0]

        class _OpDrain:
            def __next__(self):
                if not op_avail:
                    raise StopIteration
                return op_avail.popleft()

            def __iter__(self):
                return self

        op_drain = _OpDrain()

        def refill_ops(gtq, flush=False):
            while op_new and (flush or gtq - op_new[0][0] >= 2):
                op_avail.extend(op_new.popleft()[1])

        # Per-tq drain slot pattern (16 pulls max per tq; extra slots spill
        # to later tqs). qk = half-groups (4/tq), nxt = next-rep prologue
        # (6/tq during the last pair), op = outproj chunks (8/tq).
        # CRITICAL: qk (and nxt, which contains qk halves of the next rep)
        # slots come in ADJACENT PAIRS — a half-group's PSUM tile must not
        # have two other psM allocations between its half0 and half1, or
        # the 2-deep psM ring recycles it mid-accumulation.
        def std_slots(qk_it, nxt_it):
            # qk/nxt slots stay in adjacent PAIRS and even counts per tq so
            # psM half-group accumulations never have two other psM
            # allocations (or the iter-1 epilogue) between their halves.
            # 6 qk slots/tq (vs 4 needed on average) so pair0's 14-slot
            # shortfall self-heals without a flush stall at pair boundaries.
            return [qk_it, qk_it, op_drain, nxt_it, nxt_it, op_drain,
                    op_drain, qk_it, qk_it, op_drain, nxt_it, nxt_it,
                    op_drain, op_drain, nxt_it, nxt_it, op_drain, qk_it,
                    qk_it, op_drain, op_drain]

        def emit_body(carry, last):
            if carry is None:
                xTs = load_xT()
                vaugs = alloc_vaug()
                qT, kT = alloc_qk(0)
                for g in qk_groups(xTs, 0, qT, kT):
                    g[1]()
                v_iter = iter(v_items(xTs, vaugs))
                drain(v_iter, 4)  # vaug[0..3] ready before pair0/tq0 AVs
            else:
                xTs = carry["xTs"]
                vaugs = carry["vaugs"]
                qT, kT = carry["qk0"]
                v_iter = iter(())

            next_carry = {}
            nxt_iter = iter(())
            prev_outT = None
            for pair in range(MQ):
                outT = outT_pool.tile([P, NT], bf16, tag="outT", name=f"outT{pair}")
                if pair + 1 < MQ:
                    nqT, nkT = alloc_qk(pair + 1)
                    qk_iter = qk_groups(xTs, pair + 1, nqT, nkT)
                else:
                    nqT = nkT = None
                    qk_iter = iter(())
                    if not last:
                        n_xTs = load_xT()
                        n_vaugs = alloc_vaug()
                        n_qT, n_kT = alloc_qk(0)
                        nxt_iter = chain(qk_groups(n_xTs, 0, n_qT, n_kT),
                                         iter(v_items(n_xTs, n_vaugs)))
                        next_carry.update(xTs=n_xTs, vaugs=n_vaugs,
                                          qk0=(n_qT, n_kT))
                for tq in range(TQ):
                    gtq = gtq_box[0]
                    refill_ops(gtq)
                    if carry is None and pair == 0 and tq == 0:
                        slots = [v_iter] * 12 + [qk_iter] * 2
                    else:
                        slots = std_slots(qk_iter, nxt_iter)
                    if "scores" not in ablate:
                        emit_attn_tq(pair, qT, kT, vaugs, outT, tq,
                                     make_drains(slots))
                    else:
                        drain(make_drains(slots), 99)
                    if pair % 2 == 1 and "outproj" not in ablate:
                        op_new.append((gtq, [("op", pair, prev_outT, outT, tt, ec)
                                             for tt in range(tq * 4, tq * 4 + 4)
                                             for ec in range(E // 512)]))
                    gtq_box[0] += 1
                # flush any qk groups of the next pair that didn't fit in
                # this pair's drain slots: they MUST be emitted before the
                # next pair's scores read qT/kT (a missing last group left
                # qT[:, 1536:2048] uninitialized -> inf scores -> NaN)
                drain(qk_iter, 99)
                prev_outT = outT
                qT, kT = nqT, nkT
            drain(nxt_iter, 999)  # next rep's prologue leftovers
            if last:
                pump_av(0)
                refill_ops(0, flush=True)
                drain(op_drain, 999)
                drain(v_iter, 99)
            return next_carry

        carry = None
        for r in range(rep):
            carry = emit_body(carry, last=(r == rep - 1))

    nc.compile()
    return nc


MMDT = "bf16"


def _get_nc(rep=1, ablate=(), mmdt=None):
    mmdt = mmdt or MMDT
    key = ("nc", rep, tuple(sorted(ablate)), mmdt)
    if key not in _cache:
        _cache[key] = _build(rep, ablate, mmdt)
    return _cache[key]


def make_in_maps(x, w_qkv, w_out, mmdt=None):
    import ml_dtypes
    mmdt = mmdt or MMDT
    dt = {"bf16": ml_dtypes.bfloat16, "fp16": np.float16}[mmdt]
    x = np.asarray(x, dtype=np.float32).astype(dt)
    w_qkv = np.asarray(w_qkv, dtype=np.float32).astype(dt)
    w_out = np.asarray(w_out, dtype=np.float32).astype(dt)
    in_maps = []
    for c in range(N_CORES):
        b, hh = divmod(c, 2)
        hsl = slice(hh * HD, (hh + 1) * HD)
        in_maps.append({
            "xT": np.ascontiguousarray(x[b].T),
            "wqT": np.ascontiguousarray(w_qkv[0 * E:1 * E][hsl].T),
            "wkT": np.ascontiguousarray(w_qkv[1 * E:2 * E][hsl].T),
            "wvT": np.ascontiguousarray(w_qkv[2 * E:3 * E][hsl].T),
            "woT": np.ascontiguousarray(w_out[:, hsl].T),
            "ident": np.eye(P, dtype=dt),
        })
    return in_maps


def combine_outputs(results):
    y = np.empty((B, NT, E), dtype=np.float32)
    for b in range(B):
        y[b] = (results[2 * b]["y"] + results[2 * b]["y2"]
                + results[2 * b + 1]["y"] + results[2 * b + 1]["y2"])
    return y


def kernel(x, w_qkv, w_out):
    from concourse.bass_utils import run_bass_kernel_spmd
    nc = _get_nc()
    in_maps = make_in_maps(x, w_qkv, w_out)
    res = run_bass_kernel_spmd(nc, in_maps, core_ids=list(range(N_CORES)))
    return combine_outputs(res.results)
